# revision 45
# baseline (speedup 1.0000x reference)
"""Trainium2 Bass kernel for nn_Encoder_55688545960036.

Network: pointnet-style shared MLP (3->64->128, eval-mode BN folded into the
weights on the host, relu), 16 branch matmuls 128->1024 with folded BN and a
maxpool over the 2048 points of each batch element, squash over the branch
axis, capsule transform u[b,o,i,v] = sum_e caps[b,i,e] * Wc[o,i,e,v], 3 rounds
of dynamic routing, squash -> [4,32,32] output.

Distribution over 8 NeuronCores:
  phase A: branch axis k=16 -> 2 branches per core (shared MLP replicated).
  phase B: in-capsule axis i=1024 -> 128 per core (Wc 67MB -> 4MB/core bf16).
  collectives: one AllToAll that converts per-core (2 branches, all 1024 i)
  feat into per-core (all 16 branches, 128-i shard), then 3 AllReduces of the
  routing partial sums s[4,32,32] (one per routing iteration).

Key scheduling choices:
  - maxpool evacuation: each (k,oc,b) produces TWO [128,1024] psum tiles;
    either ONE DVE tensor_tensor_reduce (op0=max over the pair, op1=max
    free-axis reduce into feat) or two ACT copies to a bf16 pair tile plus a
    GpSimd tensor_scalar remax. Spreads the evacuation over DVE/ACT/Pool so
    the PE stays the bottleneck.
  - all activations (relu/square/ln/exp/copy) live in ONE act table set
    (natural_log_exp_and_others); sqrt(x) is computed as exp(0.5*ln(x)) so no
    LoadActFuncSet churn in the routing loop.
  - the branch-concat squash is folded into the capsule lhsT: the factor is
    broadcast 1->16 partitions with a ones-matmul and multiplied into the
    bf16 capsT conversion; u needs no extra scaling.
  - routing keeps (i,b) on partitions, (t=i-quarter, o, v) in free dims; the
    per-iteration squash factor is applied to the v-reduced agreement (tiny
    [128,128] op) instead of scaling s before the broadcast.
"""

import numpy as np
from contextlib import ExitStack

import concourse.bass as bass
import concourse.tile as tile
from concourse import bacc, mybir
from concourse import bass_utils

# Pin every activation to the one act-func table that holds all the funcs
# this kernel uses (exp/ln/square/relu/copy), so the table-load pass emits a
# single LoadActFuncSet instead of ping-ponging between per-func tables.
# Indices are preserved (only the *contents* of other sets are blanked), so
# the emitted act_func_set_id still matches the compiler's act_info.json.
_ACT_KEEP = "natural_log_exp_and_others"
_orig_get_act_tables = bacc.get_activation_tables
if getattr(_orig_get_act_tables, "_enc_patched", None) is None:
    def _patched_get_act_tables(arch, _orig=_orig_get_act_tables):
        t = _orig(arch)
        return {name: (funcs if name == _ACT_KEEP else set())
                for name, funcs in t.items()}
    _patched_get_act_tables._enc_patched = True
    bacc.get_activation_tables = _patched_get_act_tables

EPS = 1e-5
N_CORES = 8
B = 4
BN_ = 4 * 2048  # 8192 points
f32 = mybir.dt.float32
f32r = mybir.dt.float32r
AL = mybir.AluOpType
AF = mybir.ActivationFunctionType
AX = mybir.AxisListType
bf16 = mybir.dt.bfloat16
_BF = mybir.dt.np(bf16)

_CACHE = {}

# maxpool evacuation pattern per (k,oc,b) pair of [128,1024] psum tiles:
# D = two chained DVE tensor_scalar max-accums directly from psum
# A = two ACT copies to a bf16 pair tile + DVE 4x remax
# (the GpSimd engine cannot read PSUM and its TensorScalarPtr-with-accum is
# rejected by codegen, so the only legal evacuation engines are DVE + ACT;
# the 2:3 ratio balances their busy time)
PAT7 = ["D", "A", "A", "D", "A"]


def _build_bass(reps=1, debug=False, stage=4, nocoll=False):
    # stage: 1=MLP only, 2=+branch/maxpool, 3=+A2A/caps/u, 4=full (routing)
    # nocoll: replace collectives with local DRAM copies (for TimelineSim)
    nc = bacc.Bacc("TRN2", target_bir_lowering=False, debug=False,
                   num_devices=N_CORES)

    # ---- DRAM I/O ----
    d_xT = nc.dram_tensor("xT", [3, BN_], f32r, kind="ExternalInput").ap()
    d_w1f = nc.dram_tensor("w1f", [3, 64], f32r, kind="ExternalInput").ap()
    d_c1f = nc.dram_tensor("c1f", [64, 1], f32, kind="ExternalInput").ap()
    d_w2f = nc.dram_tensor("w2f", [64, 128], f32r, kind="ExternalInput").ap()
    d_c2f = nc.dram_tensor("c2f", [128, 1], f32, kind="ExternalInput").ap()
    d_wbT = nc.dram_tensor("wbT", [128, 2048], f32r, kind="ExternalInput").ap()
    d_cb = nc.dram_tensor("cb", [128, 16], f32, kind="ExternalInput").ap()
    d_wc = nc.dram_tensor("wc", [128, 16384], bf16, kind="ExternalInput").ap()
    d_sel132 = nc.dram_tensor("sel132", [128, 4], bf16, kind="ExternalInput").ap()
    d_sel1 = nc.dram_tensor("sel1", [128, 4], bf16, kind="ExternalInput").ap()
    d_sel4to128 = nc.dram_tensor("sel4to128", [4, 128], f32r,
                                 kind="ExternalInput").ap()
    d_ones16 = nc.dram_tensor("ones16", [16, 1], f32r, kind="ExternalInput").ap()
    d_ones1x16 = nc.dram_tensor("ones1x16", [1, 16], f32r,
                                kind="ExternalInput").ap()
    d_ident = nc.dram_tensor("ident128", [128, 128], f32,
                             kind="ExternalInput").ap()
    d_out = nc.dram_tensor("out", [B, 32, 32], f32, kind="ExternalOutput").ap()
    d_dbg = {}
    if debug:
        for nm, shp in [("h2T", [128, BN_]), ("feat", [128, 64]),
                        ("FT", [16, 512]), ("capsT", [16, 512]),
                        ("fct", [1, 512]),
                        ("lhsT", [128, 512]), ("u0", [128, 1024]),
                        ("u1", [128, 1024]), ("u2", [128, 1024]),
                        ("u3", [128, 1024]), ("blog0", [128, 128]),
                        ("sg0", [4, 1024]), ("sf0", [4, 32]),
                        ("c1it", [128, 128]), ("sg1", [4, 1024])]:
            d_dbg[nm] = nc.dram_tensor("dbg_" + nm, shp, f32,
                                       kind="ExternalOutput").ap()

    # collective bounce buffers (internal DRAM); A2A operates on first-dim
    # blocks: in[j] goes to rank j, out[r] came from rank r.
    d_a2a_in = [nc.dram_tensor(f"a2a_in_r{r}", [8, 2, B, 128], f32)
                for r in range(reps)]
    d_a2a_out = [nc.dram_tensor(f"a2a_out_r{r}", [8, 2, B, 128], f32)
                 for r in range(reps)]
    d_s_in = [[nc.dram_tensor(f"s_in{t}_r{r}", [B, 1024], f32)
               for t in range(3)] for r in range(reps)]
    d_s_out = [[nc.dram_tensor(f"s_out{t}_r{r}", [B, 1024], f32,
                               addr_space="Shared") for t in range(3)]
               for r in range(reps)]

    rg = [list(range(N_CORES))]

    with tile.TileContext(nc) as tc, ExitStack() as ctx:
        const = ctx.enter_context(tc.tile_pool(name="const", bufs=1))

        # ---- load constants / weights (all on the HWDGE queue; critical
        # path loads first, the big wc tensor last) ----
        def load_const(name, dram, shape, dt, eng=None):
            t = const.tile(shape, dt, name=name)
            (eng or nc.sync).dma_start(out=t, in_=dram)
            return t

        xT = load_const("xT_sb", d_xT, [3, BN_], f32r)
        w1f = load_const("w1f_sb", d_w1f, [3, 64], f32r)
        c1f = load_const("c1f_sb", d_c1f, [64, 1], f32)
        w2f = load_const("w2f_sb", d_w2f, [64, 128], f32r)
        c2f = load_const("c2f_sb", d_c2f, [128, 1], f32)
        wbT = load_const("wbT_sb", d_wbT, [128, 2048], f32r)
        cb = load_const("cb_sb", d_cb, [128, 16], f32)
        sel132 = load_const("sel132_sb", d_sel132, [128, 4], bf16)
        sel1 = load_const("sel1_sb", d_sel1, [128, 4], bf16)
        sel4to128 = load_const("sel4to128_sb", d_sel4to128, [4, 128], f32r)
        ones16 = load_const("ones16_sb", d_ones16, [16, 1], f32r)
        ones1x16 = load_const("ones1x16_sb", d_ones1x16, [1, 16], f32r)
        ident = load_const("ident_sb", d_ident, [128, 128], f32)
        # wc as one big [128, (g, o, v)] tile, two DMAs
        wc_sb = const.tile([128, 16384], bf16, name="wc_sb")
        for h in range(2):
            nc.sync.dma_start(out=wc_sb[:, bass.ts(h, 8192)],
                              in_=d_wc[:, bass.ts(h, 8192)])
        wc_v = wc_sb.rearrange("p (g x) -> p g x", g=16)

        def _body(rep):
            actx = ExitStack()
            pha = actx.enter_context(tc.tile_pool(name="pha", bufs=1))
            worka = actx.enter_context(tc.tile_pool(name="worka", bufs=2))
            h2T = pha.tile([128, BN_], f32r)  # [channel, point]

            # ---- phase A: shared MLP + branch matmuls + fused maxpool ----
            # ONE psum pool for the whole phase (8 banks): MLP p1/p2 take 3,
            # the branch quarter-tiles [128,512] take 5 - so branch matmuls
            # for batch b can start as soon as h2T's j=4b..4b+3 exist (the
            # b-outer loop below consumes h2T progressively).
            feat_sb = pha.tile([128, 64], f32)  # [o_in_chunk, (oc, k, b)]
            feat_pt = pha.tile([128, 64], f32)  # E-path half-group partials
            with tc.tile_pool(name="ps_a", bufs=1, space="PSUM") as ps_a:
                for j in range(16):
                    sl = bass.ts(j, 512)
                    p1 = ps_a.tile([64, 512], f32, tag="p1")
                    nc.tensor.matmul(p1, w1f, xT[:, sl], start=True, stop=True)
                    h1c = worka.tile([64, 512], f32r, tag="h1c", bufs=4)
                    # early j's gate the first branch matmuls: split the
                    # relus ACT/DVE so the serial ACT chain doesn't set the
                    # branch start time
                    if j < 4:
                        nc.vector.tensor_scalar(out=h1c, in0=p1,
                                                scalar1=c1f, scalar2=0.0,
                                                op0=AL.add, op1=AL.max)
                    else:
                        nc.scalar.activation(out=h1c, in_=p1, func=AF.Relu,
                                             bias=c1f, scale=1.0)
                    p2 = ps_a.tile([128, 512], f32, tag="p2", bufs=1)
                    nc.tensor.matmul(p2, w2f, h1c, start=True, stop=True)
                    if j % 2 == 0 or j < 4:
                        nc.scalar.activation(out=h2T[:, sl], in_=p2,
                                             func=AF.Relu, bias=c2f, scale=1.0)
                    else:
                        nc.vector.tensor_scalar(out=h2T[:, sl], in0=p2,
                                                scalar1=c2f, scalar2=0.0,
                                                op0=AL.add, op1=AL.max)

                if stage <= 1:
                    nc.sync.dma_start(
                        d_out, h2T[0:B, 0:1024].bitcast(f32)
                        .rearrange("p (o v) -> p o v", v=32))
                    actx.close()
                    return

                idx = 0
                for b in range(B):
                    for k in range(2):
                        for oc in range(8):
                            lw = wbT[:, bass.ts(k * 8 + oc, 128)]
                            py0 = ps_a.tile([128, 1024], f32, tag="py",
                                            bufs=3, name=f"py{idx}_0")
                            py1 = ps_a.tile([128, 1024], f32, tag="py",
                                            bufs=3, name=f"py{idx}_1")
                            for q in range(4):
                                dst = py0 if q < 2 else py1
                                nc.tensor.matmul(
                                    dst[:, bass.ts(q % 2, 512)], lw,
                                    h2T[:, bass.ts(4 * b + q, 512)],
                                    start=True, stop=True)
                            s = ((oc * 2 + k) * 4) + b
                            acc = feat_sb[:, s:s + 1]
                            path = PAT7[idx % len(PAT7)]
                            if path == "D":
                                # chained max-accum: 2nd op's scalar is the
                                # 1st op's partial max
                                pt = feat_pt[:, s:s + 1]
                                nc.vector.tensor_scalar(
                                    out=py0, in0=py0, scalar1=-3.0e38,
                                    scalar2=None, op0=AL.max, op1=AL.max,
                                    accum_out=pt)
                                nc.vector.tensor_scalar(
                                    out=py1, in0=py1, scalar1=pt,
                                    scalar2=None, op0=AL.max, op1=AL.max,
                                    accum_out=acc)
                            else:
                                pair = worka.tile([128, 2048], bf16,
                                                  tag="pair", bufs=4)
                                nc.scalar.copy(pair[:, 0:1024], py0)
                                nc.scalar.copy(pair[:, 1024:2048], py1)
                                nc.vector.tensor_scalar(
                                    out=pair, in0=pair, scalar1=-3.0e38,
                                    scalar2=None, op0=AL.max, op1=AL.max,
                                    accum_out=acc)
                            idx += 1

            # feat += cb (cb[p, (oc, k)] broadcast over b)
            feat_v = feat_sb.rearrange("p (oc k b) -> p oc k b", oc=8, k=2)
            cb_bc = cb.rearrange("p (oc k) -> p oc k", oc=8).unsqueeze(3) \
                      .broadcast_to((128, 8, 2, 4))
            nc.vector.tensor_add(feat_v, feat_v, cb_bc)

            if debug and rep == 0:
                nc.sync.dma_start(d_dbg["feat"], feat_sb)
            if stage <= 2:
                nc.sync.dma_start(
                    d_out[:, 0:4, 0:8],
                    feat_sb[0:B, 0:32].rearrange("p (o v) -> p o v", v=8))
                actx.close()
                return
            # transpose feat on the PE so the a2a_in DMA is one contiguous
            # 32KB copy (featT flat layout == a2a_in flat layout).
            with tc.tile_pool(name="ps_ft", bufs=1, space="PSUM") as ps_ft:
                p_ftr = ps_ft.tile([64, 128], f32, tag="pft")
                nc.tensor.transpose(p_ftr, feat_sb, ident)
                featT = worka.tile([64, 128], f32, tag="featT")
                nc.vector.tensor_copy(featT, p_ftr)
            nc.sync.dma_start(d_a2a_in[rep].ap(), featT)
            # phase-A tiles (xT, h2T, pair bufs) die here; their SBUF space
            # is reused by the phase-B pools.
            actx.close()

            # ---- AllToAll: out viewed [16(e), B, 128(i_local)] ----
            if nocoll:
                nc.sync.dma_start(d_a2a_out[rep].ap(), d_a2a_in[rep].ap())
            else:
                nc.gpsimd.collective_compute(
                    "AllToAll", AL.bypass, ins=[d_a2a_in[rep].ap().opt()],
                    outs=[d_a2a_out[rep].ap().opt()], replica_groups=rg)

            # ---- phase B ----
            with tc.tile_pool(name="ps_b", bufs=2, space="PSUM") as ps_b, \
                 tc.tile_pool(name="ps_tiny", bufs=1, space="PSUM") as ps_tiny, \
                 tc.tile_pool(name="ps_s", bufs=1, space="PSUM") as ps_s, \
                 tc.tile_pool(name="phb", bufs=1) as big, \
                 tc.tile_pool(name="workb", bufs=2) as work, \
                 tc.tile_pool(name="smallb", bufs=1) as small:

                # Keep the PE p-state ramped through the phase-B/routing
                # gaps: a pool of dependency-free matmuls at the lowest
                # scheduler priority, so the timing-driven Tile scheduler
                # slots them into PE idle stretches without ever delaying a
                # ready real matmul by more than one warmer.
                with tc.high_priority(offset=-1000000):
                    for _ in range(220):
                        pw = ps_tiny.tile([4, 512], f32, tag="warm")
                        nc.tensor.matmul(pw, sel132, wc_sb[:, 0:512],
                                         start=True, stop=True)

                # FT_bi[e, 128b + i_local] == a2a_out flat -> ONE 32KB DMA
                FT = big.tile([16, 512], f32)
                nc.sync.dma_start(
                    FT, d_a2a_out[rep].ap().rearrange("r k b l -> (r k) (b l)"))

                # squash factor fct[b,i] = |n|/(1+n^2), n2 summed over e on
                # the PE; sqrt via exp(0.5*ln) to stay in one act table.
                FT2 = work.tile([16, 512], f32r, tag="ft2")
                nc.scalar.activation(out=FT2, in_=FT, func=AF.Square,
                                     bias=0.0, scale=1.0)
                p_n2 = ps_tiny.tile([1, 512], f32, tag="pp")
                nc.tensor.matmul(p_n2, ones16, FT2, start=True, stop=True)
                lnn = small.tile([1, 512], f32, tag="lnn")
                nc.scalar.activation(out=lnn, in_=p_n2, func=AF.Ln,
                                     bias=0.0, scale=1.0)
                rt = small.tile([1, 512], f32, tag="rt")
                nc.scalar.activation(out=rt, in_=lnn, func=AF.Exp,
                                     bias=0.0, scale=0.5)
                den = small.tile([1, 512], f32, tag="den")
                nc.vector.tensor_scalar_add(den, p_n2, 1.0)
                rec = small.tile([1, 512], f32, tag="rec")
                nc.vector.reciprocal(rec, den)
                fct = small.tile([1, 512], f32r, tag="fct")
                nc.vector.tensor_mul(fct, rt, rec)
                if debug and rep == 0:
                    nc.sync.dma_start(d_dbg["fct"], fct.bitcast(f32))
                # broadcast 1 -> 16 partitions on the PE, fold into the bf16
                # capsT conversion
                p_fbc = ps_tiny.tile([16, 512], f32, tag="pp")
                nc.tensor.matmul(p_fbc, ones1x16, fct, start=True, stop=True)
                capsT = work.tile([16, 512], bf16, tag="capsT")
                nc.vector.tensor_mul(capsT, FT, p_fbc)
                if debug and rep == 0:
                    nc.sync.dma_start(d_dbg["FT"], FT)
                    nc.gpsimd.dma_start(out=d_dbg["capsT"], in_=capsT)

                # block-diagonal lhsT, g-minor column layout:
                # lhsT[16j+e, (4j+b)*16+g] = capsT[e, 128b + 16j+g]
                # -> one DMA per j with 16-element stride-1 runs.
                lhsT = big.tile([128, 512], bf16)
                nc.gpsimd.memset(lhsT, 0)
                lhsT_v = lhsT.rearrange("p (c g) -> p c g", g=16)
                capsT_v = capsT.rearrange("e (b i) -> e b i", b=4)
                for j in range(8):
                    eng = nc.sync if j % 2 == 0 else nc.gpsimd
                    eng.dma_start(
                        out=lhsT_v[16 * j:16 * (j + 1), 4 * j:4 * j + 4, :],
                        in_=capsT_v[:, :, 16 * j:16 * j + 16])

                if debug and rep == 0:
                    nc.gpsimd.dma_start(out=d_dbg["lhsT"], in_=lhsT)
                # u matmuls: group g = 4t+q holds capsules i = 16j + g;
                # lhsT operand is the stride-16 column comb [128, 32].
                # u_all[p = 32q + 4j + b, (t, o, v)] bf16.
                u_all = big.tile([128, 4096], bf16)
                for t in range(4):
                    pu = ps_b.tile([128, 1024], f32, tag="pu")
                    for q in range(4):
                        g = 4 * t + q
                        for h in range(2):
                            nc.tensor.matmul(
                                pu[32 * q:32 * q + 32, bass.ts(h, 512)],
                                lhsT_v[:, :, g],
                                wc_v[:, g, bass.ts(h, 512)],
                                start=True, stop=True,
                                tile_position=(0, 32 * q))
                    if t % 2 == 0:
                        nc.vector.tensor_copy(u_all[:, bass.ts(t, 1024)], pu)
                    else:
                        nc.scalar.copy(u_all[:, bass.ts(t, 1024)], pu)
                    if debug and rep == 0:
                        nc.gpsimd.dma_start(out=d_dbg[f"u{t}"],
                                            in_=u_all[:, bass.ts(t, 1024)])

                # ---- routing ----
                # u_all free layout is (t, v, o): o is the stride-1 dim so
                # the c/s broadcasts keep a packed last AP dim (DVE 2x mode).
                b_log = big.tile([128, 128], f32)  # [(q,j,b), (t,o)]
                uv = u_all.rearrange("p (t v o) -> p t v o", t=4, o=32)

                def s_partial(tiles, sel, pst):
                    # pst[4, 1024] = sum_t sel.T @ tiles[:, t] (partition sum
                    # selecting b = p%4); 'sel' also carries the 1/32 of c0.
                    for t in range(4):
                        for h in range(2):
                            nc.tensor.matmul(
                                pst[:, bass.ts(h, 512)], sel,
                                tiles[:, 1024 * t + 512 * h:
                                      1024 * t + 512 * (h + 1)],
                                start=(t == 0), stop=(t == 3))

                def allreduce_s(pst, it):
                    with tc.high_priority():
                        s_loc = small.tile([4, 1024], f32, tag="s_loc")
                        nc.scalar.copy(s_loc, pst)
                        nc.sync.dma_start(d_s_in[rep][it].ap(), s_loc)
                        if nocoll:
                            nc.sync.dma_start(d_s_out[rep][it].ap(),
                                              d_s_in[rep][it].ap())
                        else:
                            nc.gpsimd.collective_compute(
                                "AllReduce", AL.add,
                                ins=[d_s_in[rep][it].ap().opt()],
                                outs=[d_s_out[rep][it].ap().opt()],
                                replica_groups=rg)
                        s_glob = small.tile([4, 1024], f32r,
                                            tag=f"s_glob{it}")
                        nc.sync.dma_start(s_glob.bitcast(f32),
                                          d_s_out[rep][it].ap())
                    return s_glob

                def squash_factor(s_glob, tag):
                    # sf[b, o] = |s|/(1+|s|^2), sqrt via exp(0.5 ln); high
                    # priority so the tiny chain wins DVE/ACT queue races
                    # against the bulk agree/weight ops.
                    with tc.high_priority():
                        s2 = small.tile([4, 1024], bf16, tag="sq_s2")
                        nc.scalar.activation(out=s2,
                                             in_=s_glob.bitcast(f32),
                                             func=AF.Square,
                                             bias=0.0, scale=1.0)
                        sn2 = small.tile([4, 32], f32, tag="sq_n2")
                        nc.vector.reduce_sum(
                            sn2, s2.rearrange("p (v o) -> p o v", v=32),
                            axis=AX.X)
                        sln = small.tile([4, 32], f32, tag="sq_ln")
                        nc.scalar.activation(out=sln, in_=sn2, func=AF.Ln,
                                             bias=0.0, scale=1.0)
                        srt = small.tile([4, 32], f32, tag="sq_rt")
                        nc.scalar.activation(out=srt, in_=sln, func=AF.Exp,
                                             bias=0.0, scale=0.5)
                        sden = small.tile([4, 32], f32, tag="sq_den")
                        nc.vector.tensor_scalar_add(sden, sn2, 1.0)
                        srec = small.tile([4, 32], f32, tag="sq_rec")
                        nc.vector.reciprocal(srec, sden)
                        sf = small.tile([4, 32], f32r, tag=tag)
                        nc.vector.tensor_mul(sf, srt, srec)
                    return sf

                def agree_update(s_glob, sf, first):
                    # agree[p,(t,o)] = sf[b,o] * sum_v u[p,(t,o,v)] s[b,(o,v)]
                    # s broadcast 4->128 partitions on the PE; sf applied to
                    # the v-reduced agreement (tiny op) instead of scaling s.
                    p_sbc = ps_b.tile([128, 1024], f32, tag="pu")
                    for h in range(2):
                        nc.tensor.matmul(p_sbc[:, bass.ts(h, 512)], sel4to128,
                                         s_glob[:, bass.ts(h, 512)],
                                         start=True, stop=True)
                    with tc.high_priority():
                        p_fb = ps_tiny.tile([128, 32], f32, tag="pp")
                        nc.tensor.matmul(p_fb, sel4to128, sf,
                                         start=True, stop=True)
                    sbc = work.tile([128, 1024], bf16, tag="sbc")
                    nc.scalar.copy(sbc, p_sbc)
                    tmp = work.tile([128, 4096], bf16, tag="tmp")
                    tv = tmp.rearrange("p (t x) -> p t x", t=4)
                    nc.vector.tensor_mul(
                        tv, uv.rearrange("p t v o -> p t (v o)"),
                        sbc.unsqueeze(1).broadcast_to((128, 4, 1024)))
                    agr = work.tile([128, 128], bf16, tag="agr")
                    tr = tmp.rearrange("p (t v o) -> p t o v", t=4, o=32)
                    av = agr.rearrange("p (t o) -> p t o", o=32)
                    with nc.allow_low_precision(
                            reason="32-wide bf16 sum; b_log stays f32"):
                        nc.vector.reduce_sum(av, tr, axis=AX.X)
                    fb_bc = p_fb.unsqueeze(1).broadcast_to((128, 4, 32))
                    blv = b_log.rearrange("p (t o) -> p t o", o=32)
                    agv = agr.rearrange("p (t o) -> p t o", o=32)
                    if first:
                        nc.vector.tensor_mul(blv, agv, fb_bc)
                    else:
                        agr2 = work.tile([128, 128], f32, tag="agr2")
                        a2v = agr2.rearrange("p (t o) -> p t o", o=32)
                        nc.vector.tensor_mul(a2v, agv, fb_bc)
                        nc.vector.tensor_add(b_log, b_log, agr2)

                def softmax_c():
                    with tc.high_priority():
                        cexp = work.tile([128, 128], bf16, tag="cexp")
                        nc.scalar.activation(out=cexp, in_=b_log, func=AF.Exp,
                                             bias=0.0, scale=1.0)
                        sums = small.tile([128, 4], f32, tag="csum")
                        nc.vector.reduce_sum(
                            sums, cexp.rearrange("p (t o) -> p t o", o=32),
                            axis=AX.X)
                        crec = small.tile([128, 4], f32, tag="crec")
                        nc.vector.reciprocal(crec, sums)
                        c_sb = work.tile([128, 128], bf16, tag="c_sb")
                        nc.vector.tensor_mul(
                            c_sb.rearrange("p (t o) -> p t o", o=32),
                            cexp.rearrange("p (t o) -> p t o", o=32),
                            crec.unsqueeze(2).broadcast_to((128, 4, 32)))
                    return c_sb

                def weighted_tiles(c_sb):
                    # wt = u * c; c broadcast over the MIDDLE dim v, so the
                    # packed o stays innermost -> DVE 2x mode
                    wt = work.tile([128, 4096], bf16, tag="wt")
                    nc.vector.tensor_mul(
                        wt.rearrange("p (t v o) -> p t v o", t=4, o=32), uv,
                        c_sb.rearrange("p (t o) -> p t o", o=32)
                        .unsqueeze(2).broadcast_to((128, 4, 32, 32)))
                    return wt

                # iteration 0: c uniform = 1/32 -> s0 = sum_i u/32 on the PE
                ps0 = ps_s.tile([4, 1024], f32, tag="ps")
                s_partial(u_all, sel132, ps0)
                sg0 = allreduce_s(ps0, 0)
                if debug and rep == 0:
                    nc.sync.dma_start(d_dbg["sg0"], sg0)
                sf0 = squash_factor(sg0, "sf0")
                agree_update(sg0, sf0, first=True)
                if debug and rep == 0:
                    nc.sync.dma_start(d_dbg["sf0"], sf0)
                    nc.sync.dma_start(d_dbg["blog0"], b_log)

                # iteration 1
                c1it = softmax_c()
                if debug and rep == 0:
                    nc.sync.dma_start(d_dbg["c1it"], c1it)
                wt1 = weighted_tiles(c1it)
                ps1 = ps_s.tile([4, 1024], f32, tag="ps")
                s_partial(wt1, sel1, ps1)
                sg1 = allreduce_s(ps1, 1)
                if debug and rep == 0:
                    nc.sync.dma_start(d_dbg["sg1"], sg1)
                sf1 = squash_factor(sg1, "sf1")
                agree_update(sg1, sf1, first=False)

                # iteration 2 (final): s only, squash -> out
                wt2 = weighted_tiles(softmax_c())
                ps2 = ps_s.tile([4, 1024], f32, tag="ps")
                s_partial(wt2, sel1, ps2)
                sg2 = allreduce_s(ps2, 2)
                sf2 = squash_factor(sg2, "sf2")
                # out_sb memory is (o, v) (the d_out layout); the strided
                # write view matches sg2's (v, o) iteration order.
                out_sb = small.tile([4, 1024], f32, tag="out_sb")
                nc.vector.tensor_mul(
                    out_sb.rearrange("p (o v) -> p v o", o=32),
                    sg2.bitcast(f32).rearrange("p (v o) -> p v o", v=32),
                    sf2.bitcast(f32).unsqueeze(1).broadcast_to((4, 32, 32)))
                nc.sync.dma_start(
                    d_out, out_sb.rearrange("p (o v) -> p o v", v=32))

        for _rep in range(reps):
            _body(_rep)

    nc.compile()
    return nc


def _prepare_inputs(x, w1, g1, b1, m1, v1, w2, g2, b2, m2, v2,
                    wb, gb, bb, mb, vb, Wc):
    """Host-side: fold BN into weights, transpose/shard for the device."""
    fl = np.float32
    x = np.asarray(x, fl); w1 = np.asarray(w1, fl); w2 = np.asarray(w2, fl)
    wb = np.asarray(wb, fl); Wc = np.asarray(Wc, fl)
    g1, b1, m1, v1 = (np.asarray(a, fl) for a in (g1, b1, m1, v1))
    g2, b2, m2, v2 = (np.asarray(a, fl) for a in (g2, b2, m2, v2))
    gb, bb, mb, vb = (np.asarray(a, fl) for a in (gb, bb, mb, vb))

    s1 = g1 / np.sqrt(v1 + EPS)
    c1 = b1 - m1 * s1
    w1f = (w1 * s1[:, None]).T.copy()            # [3, 64]
    c1f = np.ascontiguousarray(c1[:, None])

    s2 = g2 / np.sqrt(v2 + EPS)
    c2 = b2 - m2 * s2
    w2f = (w2 * s2[:, None]).T.copy()            # [64, 128]
    c2f = np.ascontiguousarray(c2[:, None])

    sb = gb / np.sqrt(vb + EPS)                  # [16, 1024]
    wbp = wb * sb[:, :, None]                    # [16, 1024, 128]
    cbv = bb - mb * sb                           # [16, 1024]

    xT = np.ascontiguousarray(x.reshape(BN_, 3).T)  # [3, 8192]

    p = np.arange(128)
    sel1 = ((p[:, None] % 4) == np.arange(4)[None, :]).astype(fl)
    sel132 = sel1 / 32.0
    sel4to128 = np.ascontiguousarray(sel1.T)

    shared = {
        "xT": xT, "w1f": w1f, "c1f": c1f, "w2f": w2f, "c2f": c2f,
        "sel132": sel132.astype(_BF), "sel1": sel1.astype(_BF),
        "sel4to128": sel4to128,
        "ones16": np.ones((16, 1), fl),
        "ones1x16": np.ones((1, 16), fl),
        "ident128": np.eye(128, dtype=fl),
    }

    in_maps = []
    for c in range(N_CORES):
        m = dict(shared)
        ks = slice(2 * c, 2 * c + 2)
        # wbT[p=ch, (k, oc, o)] = wbp[2c+k, 128*oc+o, ch]
        m["wbT"] = np.ascontiguousarray(
            wbp[ks].reshape(2, 8, 128, 128).transpose(3, 0, 1, 2)
            .reshape(128, 2048))
        # cb[p, (oc, k)] = cbv[2c+k, 128*oc+p]
        m["cb"] = np.ascontiguousarray(
            cbv[ks].reshape(2, 8, 128).transpose(2, 1, 0).reshape(128, 16))
        # wc[16j+e, (g, 32v+o)] = Wc[o, 128c + 16j+g, e, v]
        # (v-major within each group so the routing's c-broadcast multiplies
        # run with a stride-1 o in the last AP dim -> DVE 2x mode)
        wcs = Wc[:, 128 * c:128 * (c + 1)]       # [32, 128, 16, 32]
        m["wc"] = np.ascontiguousarray(
            wcs.reshape(32, 8, 16, 16, 32)       # [o, j, g, e, v]
            .transpose(1, 3, 2, 4, 0)            # [j, e, g, v, o]
            .reshape(128, 16384)).astype(_BF)
        in_maps.append(m)
    return in_maps


def kernel(**inputs):
    if "nc" not in _CACHE:
        _CACHE["nc"] = _build_bass()
    nc = _CACHE["nc"]
    in_maps = _prepare_inputs(**inputs)
    res = bass_utils.run_bass_kernel_spmd(
        nc, in_maps, core_ids=list(range(N_CORES)))
    return np.asarray(res.results[0]["out"], dtype=np.float32)


# revision 50
# speedup vs baseline: 2.8448x; 2.8448x over previous
"""Trainium2 Bass kernel for nn_Encoder_55688545960036.

Network: pointnet-style shared MLP (3->64->128, eval-mode BN folded into the
weights on the host, relu), 16 branch matmuls 128->1024 with folded BN and a
maxpool over the 2048 points of each batch element, squash over the branch
axis, capsule transform u[b,o,i,v] = sum_e caps[b,i,e] * Wc[o,i,e,v], 3 rounds
of dynamic routing, squash -> [4,32,32] output.

Distribution over 8 NeuronCores:
  phase A: branch axis k=16 -> 2 branches per core (shared MLP replicated).
  phase B: in-capsule axis i=1024 -> 128 per core (Wc 67MB -> 4MB/core bf16).
  collectives: one AllToAll that converts per-core (2 branches, all 1024 i)
  feat into per-core (all 16 branches, 128-i shard), then 3 AllReduces of the
  routing partial sums s[4,32,32] (one per routing iteration).

Key scheduling choices:
  - maxpool evacuation: each (k,oc,b) produces TWO [128,1024] psum tiles;
    either ONE DVE tensor_tensor_reduce (op0=max over the pair, op1=max
    free-axis reduce into feat) or two ACT copies to a bf16 pair tile plus a
    GpSimd tensor_scalar remax. Spreads the evacuation over DVE/ACT/Pool so
    the PE stays the bottleneck.
  - all activations (relu/square/ln/exp/copy) live in ONE act table set
    (natural_log_exp_and_others); sqrt(x) is computed as exp(0.5*ln(x)) so no
    LoadActFuncSet churn in the routing loop.
  - the branch-concat squash is folded into the capsule lhsT: the factor is
    broadcast 1->16 partitions with a ones-matmul and multiplied into the
    bf16 capsT conversion; u needs no extra scaling.
  - routing keeps (i,b) on partitions, (t=i-quarter, o, v) in free dims; the
    per-iteration squash factor is applied to the v-reduced agreement (tiny
    [128,128] op) instead of scaling s before the broadcast.
"""

import numpy as np
from contextlib import ExitStack

import concourse.bass as bass
import concourse.tile as tile
from concourse import bacc, mybir
from concourse import bass_utils

# Pin every activation to the one act-func table that holds all the funcs
# this kernel uses (exp/ln/square/relu/copy), so the table-load pass emits a
# single LoadActFuncSet instead of ping-ponging between per-func tables.
# Indices are preserved (only the *contents* of other sets are blanked), so
# the emitted act_func_set_id still matches the compiler's act_info.json.
_ACT_KEEP = "natural_log_exp_and_others"
_orig_get_act_tables = bacc.get_activation_tables
if getattr(_orig_get_act_tables, "_enc_patched", None) is None:
    def _patched_get_act_tables(arch, _orig=_orig_get_act_tables):
        t = _orig(arch)
        return {name: (funcs if name == _ACT_KEEP else set())
                for name, funcs in t.items()}
    _patched_get_act_tables._enc_patched = True
    bacc.get_activation_tables = _patched_get_act_tables

EPS = 1e-5
N_CORES = 8
B = 4
BN_ = 4 * 2048  # 8192 points
f32 = mybir.dt.float32
f32r = mybir.dt.float32r
AL = mybir.AluOpType
AF = mybir.ActivationFunctionType
AX = mybir.AxisListType
bf16 = mybir.dt.bfloat16
_BF = mybir.dt.np(bf16)

_CACHE = {}

# maxpool evacuation pattern per (k,oc,b) pair of [128,1024] psum tiles:
# D = two chained DVE tensor_scalar max-accums directly from psum
# A = two ACT copies to a bf16 pair tile + DVE 4x remax
# (the GpSimd engine cannot read PSUM and its TensorScalarPtr-with-accum is
# rejected by codegen, so the only legal evacuation engines are DVE + ACT;
# the 2:3 ratio balances their busy time)
PAT7 = ["D", "A", "A", "D", "A"]


def _build_bass(reps=1, debug=False, stage=4, nocoll=False):
    # stage: 1=MLP only, 2=+branch/maxpool, 3=+A2A/caps/u, 4=full (routing)
    # nocoll: replace collectives with local DRAM copies (for TimelineSim)
    nc = bacc.Bacc("TRN2", target_bir_lowering=False, debug=False,
                   num_devices=N_CORES)

    # ---- DRAM I/O ----
    d_xT = nc.dram_tensor("xT", [3, BN_], f32r, kind="ExternalInput").ap()
    d_w1f = nc.dram_tensor("w1f", [3, 64], f32r, kind="ExternalInput").ap()
    d_c1f = nc.dram_tensor("c1f", [64, 1], f32, kind="ExternalInput").ap()
    d_w2f = nc.dram_tensor("w2f", [64, 128], f32r, kind="ExternalInput").ap()
    d_c2f = nc.dram_tensor("c2f", [128, 1], f32, kind="ExternalInput").ap()
    d_wbT = nc.dram_tensor("wbT", [128, 2048], f32r, kind="ExternalInput").ap()
    d_cb = nc.dram_tensor("cb", [128, 16], f32, kind="ExternalInput").ap()
    d_wc = nc.dram_tensor("wc", [128, 16384], bf16, kind="ExternalInput").ap()
    d_sel132 = nc.dram_tensor("sel132", [128, 4], bf16, kind="ExternalInput").ap()
    d_sel1 = nc.dram_tensor("sel1", [128, 4], bf16, kind="ExternalInput").ap()
    d_sel4to128 = nc.dram_tensor("sel4to128", [4, 128], f32r,
                                 kind="ExternalInput").ap()
    d_ones16 = nc.dram_tensor("ones16", [16, 1], f32r, kind="ExternalInput").ap()
    d_ones1x16 = nc.dram_tensor("ones1x16", [1, 16], f32r,
                                kind="ExternalInput").ap()
    d_ident = nc.dram_tensor("ident128", [128, 128], f32,
                             kind="ExternalInput").ap()
    d_out = nc.dram_tensor("out", [B, 32, 32], f32, kind="ExternalOutput").ap()
    d_dbg = {}
    if debug:
        for nm, shp in [("h2T", [128, BN_]), ("feat", [128, 64]),
                        ("FT", [16, 512]), ("capsT", [16, 512]),
                        ("fct", [1, 512]),
                        ("lhsT", [128, 512]), ("u0", [128, 1024]),
                        ("u1", [128, 1024]), ("u2", [128, 1024]),
                        ("u3", [128, 1024]), ("blog0", [128, 128]),
                        ("sg0", [4, 1024]), ("sf0", [4, 32]),
                        ("c1it", [128, 128]), ("sg1", [4, 1024])]:
            d_dbg[nm] = nc.dram_tensor("dbg_" + nm, shp, f32,
                                       kind="ExternalOutput").ap()

    # collective bounce buffers (internal DRAM); A2A operates on first-dim
    # blocks: in[j] goes to rank j, out[r] came from rank r.
    d_a2a_in = [nc.dram_tensor(f"a2a_in_r{r}", [8, 2, B, 128], f32)
                for r in range(reps)]
    d_a2a_out = [nc.dram_tensor(f"a2a_out_r{r}", [8, 2, B, 128], f32)
                 for r in range(reps)]
    d_s_in = [[nc.dram_tensor(f"s_in{t}_r{r}", [B, 1024], f32)
               for t in range(3)] for r in range(reps)]
    d_s_out = [[nc.dram_tensor(f"s_out{t}_r{r}", [B, 1024], f32,
                               addr_space="Shared") for t in range(3)]
               for r in range(reps)]

    rg = [list(range(N_CORES))]

    with tile.TileContext(nc) as tc, ExitStack() as ctx:
        const = ctx.enter_context(tc.tile_pool(name="const", bufs=1))

        # ---- load constants / weights (all on the HWDGE queue; critical
        # path loads first, the big wc tensor last) ----
        def load_const(name, dram, shape, dt, eng=None):
            t = const.tile(shape, dt, name=name)
            (eng or nc.sync).dma_start(out=t, in_=dram)
            return t

        xT = load_const("xT_sb", d_xT, [3, BN_], f32r)
        w1f = load_const("w1f_sb", d_w1f, [3, 64], f32r)
        c1f = load_const("c1f_sb", d_c1f, [64, 1], f32)
        w2f = load_const("w2f_sb", d_w2f, [64, 128], f32r)
        c2f = load_const("c2f_sb", d_c2f, [128, 1], f32)
        wbT = load_const("wbT_sb", d_wbT, [128, 2048], f32r)
        cb = load_const("cb_sb", d_cb, [128, 16], f32)
        sel132 = load_const("sel132_sb", d_sel132, [128, 4], bf16)
        sel1 = load_const("sel1_sb", d_sel1, [128, 4], bf16)
        sel4to128 = load_const("sel4to128_sb", d_sel4to128, [4, 128], f32r)
        ones16 = load_const("ones16_sb", d_ones16, [16, 1], f32r)
        ones1x16 = load_const("ones1x16_sb", d_ones1x16, [1, 16], f32r)
        ident = load_const("ident_sb", d_ident, [128, 128], f32)
        # wc as one big [128, (g, o, v)] tile, two DMAs
        wc_sb = const.tile([128, 16384], bf16, name="wc_sb")
        for h in range(2):
            nc.sync.dma_start(out=wc_sb[:, bass.ts(h, 8192)],
                              in_=d_wc[:, bass.ts(h, 8192)])
        wc_v = wc_sb.rearrange("p (g x) -> p g x", g=16)

        def _body(rep):
            actx = ExitStack()
            pha = actx.enter_context(tc.tile_pool(name="pha", bufs=1))
            worka = actx.enter_context(tc.tile_pool(name="worka", bufs=2))
            h2T = pha.tile([128, BN_], f32r)  # [channel, point]

            # ---- phase A: shared MLP + branch matmuls + fused maxpool ----
            # ONE psum pool for the whole phase (8 banks): MLP p1/p2 take 3,
            # the branch quarter-tiles [128,512] take 5 - so branch matmuls
            # for batch b can start as soon as h2T's j=4b..4b+3 exist (the
            # b-outer loop below consumes h2T progressively).
            feat_sb = pha.tile([128, 64], f32)  # [o_in_chunk, (oc, k, b)]
            feat_pt = pha.tile([128, 64], f32)  # E-path half-group partials
            with tc.tile_pool(name="ps_a", bufs=1, space="PSUM") as ps_a:
                for j in range(16):
                    sl = bass.ts(j, 512)
                    p1 = ps_a.tile([64, 512], f32, tag="p1")
                    nc.tensor.matmul(p1, w1f, xT[:, sl], start=True, stop=True)
                    h1c = worka.tile([64, 512], f32r, tag="h1c", bufs=4)
                    # early j's gate the first branch matmuls: split the
                    # relus ACT/DVE so the serial ACT chain doesn't set the
                    # branch start time
                    if j < 8:
                        nc.vector.tensor_scalar(out=h1c, in0=p1,
                                                scalar1=c1f, scalar2=0.0,
                                                op0=AL.add, op1=AL.max)
                    else:
                        nc.scalar.activation(out=h1c, in_=p1, func=AF.Relu,
                                             bias=c1f, scale=1.0)
                    p2 = ps_a.tile([128, 512], f32, tag="p2", bufs=1)
                    nc.tensor.matmul(p2, w2f, h1c, start=True, stop=True)
                    if j % 2 == 0 or j < 4:
                        nc.scalar.activation(out=h2T[:, sl], in_=p2,
                                             func=AF.Relu, bias=c2f, scale=1.0)
                    else:
                        nc.vector.tensor_scalar(out=h2T[:, sl], in0=p2,
                                                scalar1=c2f, scalar2=0.0,
                                                op0=AL.add, op1=AL.max)

                if stage <= 1:
                    nc.sync.dma_start(
                        d_out, h2T[0:B, 0:1024].bitcast(f32)
                        .rearrange("p (o v) -> p o v", v=32))
                    actx.close()
                    return

                idx = 0
                for b in range(B):
                  for k in range(2):
                    for oc in range(8):
                        lw = wbT[:, bass.ts(k * 8 + oc, 128)]
                        if True:
                            py0 = ps_a.tile([128, 1024], f32, tag="py",
                                            bufs=3, name=f"py{idx}_0")
                            py1 = ps_a.tile([128, 1024], f32, tag="py",
                                            bufs=3, name=f"py{idx}_1")
                            for q in range(4):
                                dst = py0 if q < 2 else py1
                                nc.tensor.matmul(
                                    dst[:, bass.ts(q % 2, 512)], lw,
                                    h2T[:, bass.ts(4 * b + q, 512)],
                                    start=True, stop=True)
                            s = ((oc * 2 + k) * 4) + b
                            acc = feat_sb[:, s:s + 1]
                            path = PAT7[idx % len(PAT7)]
                            if path == "D":
                                # chained max-accum: 2nd op's scalar is the
                                # 1st op's partial max
                                pt = feat_pt[:, s:s + 1]
                                nc.vector.tensor_scalar(
                                    out=py0, in0=py0, scalar1=-3.0e38,
                                    scalar2=None, op0=AL.max, op1=AL.max,
                                    accum_out=pt)
                                nc.vector.tensor_scalar(
                                    out=py1, in0=py1, scalar1=pt,
                                    scalar2=None, op0=AL.max, op1=AL.max,
                                    accum_out=acc)
                            else:
                                pair = worka.tile([128, 2048], bf16,
                                                  tag="pair", bufs=4)
                                nc.scalar.copy(pair[:, 0:1024], py0)
                                nc.scalar.copy(pair[:, 1024:2048], py1)
                                nc.vector.tensor_scalar(
                                    out=pair, in0=pair, scalar1=-3.0e38,
                                    scalar2=None, op0=AL.max, op1=AL.max,
                                    accum_out=acc)
                            idx += 1

            # feat += cb (cb[p, (oc, k)] broadcast over b)
            feat_v = feat_sb.rearrange("p (oc k b) -> p oc k b", oc=8, k=2)
            cb_bc = cb.rearrange("p (oc k) -> p oc k", oc=8).unsqueeze(3) \
                      .broadcast_to((128, 8, 2, 4))
            nc.vector.tensor_add(feat_v, feat_v, cb_bc)

            if debug and rep == 0:
                nc.sync.dma_start(d_dbg["feat"], feat_sb)
            if stage <= 2:
                nc.sync.dma_start(
                    d_out[:, 0:4, 0:8],
                    feat_sb[0:B, 0:32].rearrange("p (o v) -> p o v", v=8))
                actx.close()
                return
            # transpose feat on the PE so the a2a_in DMA is one contiguous
            # 32KB copy (featT flat layout == a2a_in flat layout).
            with tc.tile_pool(name="ps_ft", bufs=1, space="PSUM") as ps_ft:
                p_ftr = ps_ft.tile([64, 128], f32, tag="pft")
                nc.tensor.transpose(p_ftr, feat_sb, ident)
                featT = worka.tile([64, 128], f32, tag="featT")
                nc.vector.tensor_copy(featT, p_ftr)
            nc.sync.dma_start(d_a2a_in[rep].ap(), featT)
            # phase-A tiles (xT, h2T, pair bufs) die here; their SBUF space
            # is reused by the phase-B pools.
            actx.close()

            # ---- AllToAll: out viewed [16(e), B, 128(i_local)] ----
            if nocoll:
                nc.sync.dma_start(d_a2a_out[rep].ap(), d_a2a_in[rep].ap())
            else:
                nc.gpsimd.collective_compute(
                    "AllToAll", AL.bypass, ins=[d_a2a_in[rep].ap().opt()],
                    outs=[d_a2a_out[rep].ap().opt()], replica_groups=rg)

            # ---- phase B ----
            with tc.tile_pool(name="ps_b", bufs=2, space="PSUM") as ps_b, \
                 tc.tile_pool(name="ps_tiny", bufs=1, space="PSUM") as ps_tiny, \
                 tc.tile_pool(name="ps_s", bufs=1, space="PSUM") as ps_s, \
                 tc.tile_pool(name="phb", bufs=1) as big, \
                 tc.tile_pool(name="workb", bufs=2) as work, \
                 tc.tile_pool(name="smallb", bufs=1) as small:

                # Keep the PE p-state ramped through the phase-B/routing
                # gaps: a pool of dependency-free matmuls at the lowest
                # scheduler priority, so the timing-driven Tile scheduler
                # slots them into PE idle stretches without ever delaying a
                # ready real matmul by more than one warmer.
                with tc.high_priority(offset=-1000000):
                    for _ in range(220):
                        pw = ps_tiny.tile([4, 512], f32, tag="warm")
                        nc.tensor.matmul(pw, sel132, wc_sb[:, 0:512],
                                         start=True, stop=True)

                # FT_bi[e, 128b + i_local] == a2a_out flat -> ONE 32KB DMA
                FT = big.tile([16, 512], f32)
                nc.sync.dma_start(
                    FT, d_a2a_out[rep].ap().rearrange("r k b l -> (r k) (b l)"))

                # squash factor fct[b,i] = |n|/(1+n^2), n2 summed over e on
                # the PE; sqrt via exp(0.5*ln) to stay in one act table.
                FT2 = work.tile([16, 512], f32r, tag="ft2")
                nc.scalar.activation(out=FT2, in_=FT, func=AF.Square,
                                     bias=0.0, scale=1.0)
                p_n2 = ps_tiny.tile([1, 512], f32, tag="pp")
                nc.tensor.matmul(p_n2, ones16, FT2, start=True, stop=True)
                lnn = small.tile([1, 512], f32, tag="lnn")
                nc.scalar.activation(out=lnn, in_=p_n2, func=AF.Ln,
                                     bias=0.0, scale=1.0)
                ln1 = small.tile([1, 512], f32, tag="ln1")
                nc.scalar.activation(out=ln1, in_=p_n2, func=AF.Ln,
                                     bias=1.0, scale=1.0)
                fd = small.tile([1, 512], f32, tag="fd")
                nc.vector.scalar_tensor_tensor(
                    out=fd, in0=lnn, scalar=0.5, in1=ln1,
                    op0=AL.mult, op1=AL.subtract)
                fct = small.tile([1, 512], f32r, tag="fct")
                nc.scalar.activation(out=fct, in_=fd, func=AF.Exp,
                                     bias=0.0, scale=1.0)
                if debug and rep == 0:
                    nc.sync.dma_start(d_dbg["fct"], fct.bitcast(f32))
                # broadcast 1 -> 16 partitions on the PE, fold into the bf16
                # capsT conversion
                p_fbc = ps_tiny.tile([16, 512], f32, tag="pp")
                nc.tensor.matmul(p_fbc, ones1x16, fct, start=True, stop=True)
                capsT = work.tile([16, 512], bf16, tag="capsT")
                nc.vector.tensor_mul(capsT, FT, p_fbc)
                if debug and rep == 0:
                    nc.sync.dma_start(d_dbg["FT"], FT)
                    nc.gpsimd.dma_start(out=d_dbg["capsT"], in_=capsT)

                # block-diagonal lhsT, g-minor column layout:
                # lhsT[16j+e, (4j+b)*16+g] = capsT[e, 128b + 16j+g]
                # -> one DMA per j with 16-element stride-1 runs.
                lhsT = big.tile([128, 512], bf16)
                nc.gpsimd.memset(lhsT, 0)
                lhsT_v = lhsT.rearrange("p (c g) -> p c g", g=16)
                capsT_v = capsT.rearrange("e (b i) -> e b i", b=4)
                for j in range(8):
                    eng = nc.sync if j % 2 == 0 else nc.gpsimd
                    eng.dma_start(
                        out=lhsT_v[16 * j:16 * (j + 1), 4 * j:4 * j + 4, :],
                        in_=capsT_v[:, :, 16 * j:16 * j + 16])

                if debug and rep == 0:
                    nc.gpsimd.dma_start(out=d_dbg["lhsT"], in_=lhsT)
                # u matmuls: group g = 4t+q holds capsules i = 16j + g;
                # lhsT operand is the stride-16 column comb [128, 32].
                # u_all[p = 32q + 4j + b, (t, o, v)] bf16.
                u_all = big.tile([128, 4096], bf16)
                for t in range(4):
                    pu = ps_b.tile([128, 1024], f32, tag="pu")
                    for q in range(4):
                        g = 4 * t + q
                        for h in range(2):
                            nc.tensor.matmul(
                                pu[32 * q:32 * q + 32, bass.ts(h, 512)],
                                lhsT_v[:, :, g],
                                wc_v[:, g, bass.ts(h, 512)],
                                start=True, stop=True,
                                tile_position=(0, 32 * q))
                    if t % 2 == 0:
                        nc.vector.tensor_copy(u_all[:, bass.ts(t, 1024)], pu)
                    else:
                        nc.scalar.copy(u_all[:, bass.ts(t, 1024)], pu)
                    if debug and rep == 0:
                        nc.gpsimd.dma_start(out=d_dbg[f"u{t}"],
                                            in_=u_all[:, bass.ts(t, 1024)])

                # ---- routing ----
                # u_all free layout is (t, v, o): o is the stride-1 dim so
                # the c/s broadcasts keep a packed last AP dim (DVE 2x mode).
                b_log = big.tile([128, 128], f32)  # [(q,j,b), (t,o)]
                uv = u_all.rearrange("p (t v o) -> p t v o", t=4, o=32)

                def s_partial(tiles, sel, pst):
                    # pst[4, 1024] = sum_t sel.T @ tiles[:, t] (partition sum
                    # selecting b = p%4); 'sel' also carries the 1/32 of c0.
                    for t in range(4):
                        for h in range(2):
                            nc.tensor.matmul(
                                pst[:, bass.ts(h, 512)], sel,
                                tiles[:, 1024 * t + 512 * h:
                                      1024 * t + 512 * (h + 1)],
                                start=(t == 0), stop=(t == 3))

                def allreduce_s(pst, it):
                    with tc.high_priority():
                        s_loc = small.tile([4, 1024], f32, tag="s_loc")
                        nc.scalar.copy(s_loc, pst)
                        nc.sync.dma_start(d_s_in[rep][it].ap(), s_loc)
                        if nocoll:
                            nc.sync.dma_start(d_s_out[rep][it].ap(),
                                              d_s_in[rep][it].ap())
                        else:
                            nc.gpsimd.collective_compute(
                                "AllReduce", AL.add,
                                ins=[d_s_in[rep][it].ap().opt()],
                                outs=[d_s_out[rep][it].ap().opt()],
                                replica_groups=rg)
                        s_glob = small.tile([4, 1024], f32r,
                                            tag=f"s_glob{it}")
                        nc.sync.dma_start(s_glob.bitcast(f32),
                                          d_s_out[rep][it].ap())
                    return s_glob

                def squash_factor(s_glob, tag):
                    # sf[b, o] = |s|/(1+|s|^2), sqrt via exp(0.5 ln); high
                    # priority so the tiny chain wins DVE/ACT queue races
                    # against the bulk agree/weight ops.
                    with tc.high_priority():
                        s2 = small.tile([4, 1024], bf16, tag="sq_s2")
                        nc.scalar.activation(out=s2,
                                             in_=s_glob.bitcast(f32),
                                             func=AF.Square,
                                             bias=0.0, scale=1.0)
                        sn2 = small.tile([4, 32], f32, tag="sq_n2")
                        nc.vector.reduce_sum(
                            sn2, s2.rearrange("p (v o) -> p o v", v=32),
                            axis=AX.X)
                        # f = sqrt(n2)/(1+n2) = exp(0.5 ln n2 - ln(1+n2))
                        sln = small.tile([4, 32], f32, tag="sq_ln")
                        nc.scalar.activation(out=sln, in_=sn2, func=AF.Ln,
                                             bias=0.0, scale=1.0)
                        sl1 = small.tile([4, 32], f32, tag="sq_l1")
                        nc.scalar.activation(out=sl1, in_=sn2, func=AF.Ln,
                                             bias=1.0, scale=1.0)
                        sd = small.tile([4, 32], f32, tag="sq_d")
                        nc.vector.scalar_tensor_tensor(
                            out=sd, in0=sln, scalar=0.5, in1=sl1,
                            op0=AL.mult, op1=AL.subtract)
                        sf = small.tile([4, 32], f32r, tag=tag)
                        nc.scalar.activation(out=sf, in_=sd, func=AF.Exp,
                                             bias=0.0, scale=1.0)
                    return sf

                def agree_update(s_glob, sf, first):
                    # agree[p,(t,o)] = sf[b,o] * sum_v u[p,(t,o,v)] s[b,(o,v)]
                    # s broadcast 4->128 partitions on the PE; sf applied to
                    # the v-reduced agreement (tiny op) instead of scaling s.
                    p_sbc = ps_b.tile([128, 1024], f32, tag="pu")
                    for h in range(2):
                        nc.tensor.matmul(p_sbc[:, bass.ts(h, 512)], sel4to128,
                                         s_glob[:, bass.ts(h, 512)],
                                         start=True, stop=True)
                    with tc.high_priority():
                        p_fb = ps_tiny.tile([128, 32], f32, tag="pp")
                        nc.tensor.matmul(p_fb, sel4to128, sf,
                                         start=True, stop=True)
                    sbc = work.tile([128, 1024], bf16, tag="sbc")
                    nc.scalar.copy(sbc, p_sbc)
                    tmp = work.tile([128, 4096], bf16, tag="tmp")
                    tv = tmp.rearrange("p (t x) -> p t x", t=4)
                    nc.vector.tensor_mul(
                        tv, uv.rearrange("p t v o -> p t (v o)"),
                        sbc.unsqueeze(1).broadcast_to((128, 4, 1024)))
                    # v-reduction as a TT fold tree: TensorTensor runs at
                    # 2x in bf16 (packed o innermost) while TensorReduce has
                    # no fast mode.
                    scr = work.tile([128, 2048], bf16, tag="fold")
                    tv4 = tmp.rearrange("p (t v o) -> p t v o", t=4, o=32)
                    sv = scr.rearrange("p (t v o) -> p t v o", t=4, o=32)
                    nc.vector.tensor_add(sv[:, :, 0:16], tv4[:, :, 0:16],
                                         tv4[:, :, 16:32])
                    nc.vector.tensor_add(sv[:, :, 0:8], sv[:, :, 0:8],
                                         sv[:, :, 8:16])
                    nc.vector.tensor_add(sv[:, :, 0:4], sv[:, :, 0:4],
                                         sv[:, :, 4:8])
                    nc.vector.tensor_add(sv[:, :, 0:2], sv[:, :, 0:2],
                                         sv[:, :, 2:4])
                    agr = work.tile([128, 128], bf16, tag="agr")
                    av = agr.rearrange("p (t o) -> p t o", o=32)
                    nc.vector.tensor_add(av, sv[:, :, 0, :], sv[:, :, 1, :])
                    fb_bc = p_fb.unsqueeze(1).broadcast_to((128, 4, 32))
                    blv = b_log.rearrange("p (t o) -> p t o", o=32)
                    agv = agr.rearrange("p (t o) -> p t o", o=32)
                    if first:
                        nc.vector.tensor_mul(blv, agv, fb_bc)
                    else:
                        agr2 = work.tile([128, 128], f32, tag="agr2")
                        a2v = agr2.rearrange("p (t o) -> p t o", o=32)
                        nc.vector.tensor_mul(a2v, agv, fb_bc)
                        nc.vector.tensor_add(b_log, b_log, agr2)

                def softmax_c():
                    with tc.high_priority():
                        cexp = work.tile([128, 128], bf16, tag="cexp")
                        nc.scalar.activation(out=cexp, in_=b_log, func=AF.Exp,
                                             bias=0.0, scale=1.0)
                        sums = small.tile([128, 4], f32, tag="csum")
                        nc.vector.reduce_sum(
                            sums, cexp.rearrange("p (t o) -> p t o", o=32),
                            axis=AX.X)
                        crec = small.tile([128, 4], f32, tag="crec")
                        nc.vector.reciprocal(crec, sums)
                        c_sb = work.tile([128, 128], bf16, tag="c_sb")
                        nc.vector.tensor_mul(
                            c_sb.rearrange("p (t o) -> p t o", o=32),
                            cexp.rearrange("p (t o) -> p t o", o=32),
                            crec.unsqueeze(2).broadcast_to((128, 4, 32)))
                    return c_sb

                def weighted_tiles(c_sb):
                    # wt = u * c; c broadcast over the MIDDLE dim v, so the
                    # packed o stays innermost -> DVE 2x mode
                    wt = work.tile([128, 4096], bf16, tag="wt")
                    nc.vector.tensor_mul(
                        wt.rearrange("p (t v o) -> p t v o", t=4, o=32), uv,
                        c_sb.rearrange("p (t o) -> p t o", o=32)
                        .unsqueeze(2).broadcast_to((128, 4, 32, 32)))
                    return wt

                # iteration 0: c uniform = 1/32 -> s0 = sum_i u/32 on the PE
                ps0 = ps_s.tile([4, 1024], f32, tag="ps")
                s_partial(u_all, sel132, ps0)
                sg0 = allreduce_s(ps0, 0)
                if debug and rep == 0:
                    nc.sync.dma_start(d_dbg["sg0"], sg0)
                sf0 = squash_factor(sg0, "sf0")
                agree_update(sg0, sf0, first=True)
                if debug and rep == 0:
                    nc.sync.dma_start(d_dbg["sf0"], sf0)
                    nc.sync.dma_start(d_dbg["blog0"], b_log)

                # iteration 1
                c1it = softmax_c()
                if debug and rep == 0:
                    nc.sync.dma_start(d_dbg["c1it"], c1it)
                wt1 = weighted_tiles(c1it)
                ps1 = ps_s.tile([4, 1024], f32, tag="ps")
                s_partial(wt1, sel1, ps1)
                sg1 = allreduce_s(ps1, 1)
                if debug and rep == 0:
                    nc.sync.dma_start(d_dbg["sg1"], sg1)
                sf1 = squash_factor(sg1, "sf1")
                agree_update(sg1, sf1, first=False)

                # iteration 2 (final): s only, squash -> out
                wt2 = weighted_tiles(softmax_c())
                ps2 = ps_s.tile([4, 1024], f32, tag="ps")
                s_partial(wt2, sel1, ps2)
                sg2 = allreduce_s(ps2, 2)
                sf2 = squash_factor(sg2, "sf2")
                # out_sb memory is (o, v) (the d_out layout); the strided
                # write view matches sg2's (v, o) iteration order.
                out_sb = small.tile([4, 1024], f32, tag="out_sb")
                nc.vector.tensor_mul(
                    out_sb.rearrange("p (o v) -> p v o", o=32),
                    sg2.bitcast(f32).rearrange("p (v o) -> p v o", v=32),
                    sf2.bitcast(f32).unsqueeze(1).broadcast_to((4, 32, 32)))
                nc.sync.dma_start(
                    d_out, out_sb.rearrange("p (o v) -> p o v", v=32))

        for _rep in range(reps):
            _body(_rep)

    nc.compile()
    return nc


def _prepare_inputs(x, w1, g1, b1, m1, v1, w2, g2, b2, m2, v2,
                    wb, gb, bb, mb, vb, Wc):
    """Host-side: fold BN into weights, transpose/shard for the device."""
    fl = np.float32
    x = np.asarray(x, fl); w1 = np.asarray(w1, fl); w2 = np.asarray(w2, fl)
    wb = np.asarray(wb, fl); Wc = np.asarray(Wc, fl)
    g1, b1, m1, v1 = (np.asarray(a, fl) for a in (g1, b1, m1, v1))
    g2, b2, m2, v2 = (np.asarray(a, fl) for a in (g2, b2, m2, v2))
    gb, bb, mb, vb = (np.asarray(a, fl) for a in (gb, bb, mb, vb))

    s1 = g1 / np.sqrt(v1 + EPS)
    c1 = b1 - m1 * s1
    w1f = (w1 * s1[:, None]).T.copy()            # [3, 64]
    c1f = np.ascontiguousarray(c1[:, None])

    s2 = g2 / np.sqrt(v2 + EPS)
    c2 = b2 - m2 * s2
    w2f = (w2 * s2[:, None]).T.copy()            # [64, 128]
    c2f = np.ascontiguousarray(c2[:, None])

    sb = gb / np.sqrt(vb + EPS)                  # [16, 1024]
    wbp = wb * sb[:, :, None]                    # [16, 1024, 128]
    cbv = bb - mb * sb                           # [16, 1024]

    xT = np.ascontiguousarray(x.reshape(BN_, 3).T)  # [3, 8192]

    p = np.arange(128)
    sel1 = ((p[:, None] % 4) == np.arange(4)[None, :]).astype(fl)
    sel132 = sel1 / 32.0
    sel4to128 = np.ascontiguousarray(sel1.T)

    shared = {
        "xT": xT, "w1f": w1f, "c1f": c1f, "w2f": w2f, "c2f": c2f,
        "sel132": sel132.astype(_BF), "sel1": sel1.astype(_BF),
        "sel4to128": sel4to128,
        "ones16": np.ones((16, 1), fl),
        "ones1x16": np.ones((1, 16), fl),
        "ident128": np.eye(128, dtype=fl),
    }

    in_maps = []
    for c in range(N_CORES):
        m = dict(shared)
        ks = slice(2 * c, 2 * c + 2)
        # wbT[p=ch, (k, oc, o)] = wbp[2c+k, 128*oc+o, ch]
        m["wbT"] = np.ascontiguousarray(
            wbp[ks].reshape(2, 8, 128, 128).transpose(3, 0, 1, 2)
            .reshape(128, 2048))
        # cb[p, (oc, k)] = cbv[2c+k, 128*oc+p]
        m["cb"] = np.ascontiguousarray(
            cbv[ks].reshape(2, 8, 128).transpose(2, 1, 0).reshape(128, 16))
        # wc[16j+e, (g, 32v+o)] = Wc[o, 128c + 16j+g, e, v]
        # (v-major within each group so the routing's c-broadcast multiplies
        # run with a stride-1 o in the last AP dim -> DVE 2x mode)
        wcs = Wc[:, 128 * c:128 * (c + 1)]       # [32, 128, 16, 32]
        m["wc"] = np.ascontiguousarray(
            wcs.reshape(32, 8, 16, 16, 32)       # [o, j, g, e, v]
            .transpose(1, 3, 2, 4, 0)            # [j, e, g, v, o]
            .reshape(128, 16384)).astype(_BF)
        in_maps.append(m)
    return in_maps


def kernel(**inputs):
    if "nc" not in _CACHE:
        _CACHE["nc"] = _build_bass()
    nc = _CACHE["nc"]
    in_maps = _prepare_inputs(**inputs)
    res = bass_utils.run_bass_kernel_spmd(
        nc, in_maps, core_ids=list(range(N_CORES)))
    return np.asarray(res.results[0]["out"], dtype=np.float32)
